# revision 19
# baseline (speedup 1.0000x reference)
"""Trainium2 Bass kernel: sparse 7x7x7 stride-1 max-pool over a 64^3 voxel grid
(MinkowskiEngine semantics) + per-point MLP (1x1 conv -> ReLU -> 1x1 conv ->
sigmoid) * feats.

v2 strategy (8 NeuronCores, SPMD, no collectives):
  - Host pre-builds, per core, a dense z-slab grid in *pooling layout*:
    PG[p=128, x, h, y, z] bf16 where channel c = h*128+p, y padded to 70
    (3 each side, -inf), z = 16 (14-slab for the 7-window + 2 -inf pad so
    every DVE windowed-max op has an even inner count). Core k owns
    z in [8k, 8k+8).
  - Device loop over 64 x-planes: direct DMA of the plane (no scatter, no
    transpose), separable windowed max (7 = (2,4;3) -> 3 tensor_max per
    axis) split across DVE and GpSimd, fused MLP on PE
    (h = relu(W1.T @ px) then y2 = W2.T-halves @ h -> [C, vox] layout),
    sigmoid on ACT, plane written to the dense sig output grid.
  - Host gathers the per-point sig rows from the dense output grids and
    multiplies by the exact fp32 feats rows (cheap elementwise epilogue).

All pooling numerics identical to v1 (bf16 max is order-preserving).
"""

from contextlib import ExitStack
from dataclasses import dataclass, field

import numpy as np

C = 256
R = 128
D = 64
NX = 64
ZS = 8            # owned z-planes per core
ZH = 14           # z slab incl 3+3 halo
ZP = 16           # z padded (even inner counts)
YP = 70           # y padded 3+3
NPTS = 100000
SENT = -1.0e30
PLANE = 2 * YP * ZP      # free elems per plane per partition (4480 B bf16)
VOXH = D * ZS            # owned voxels per x-plane (512)
VOX2 = 2 * VOXH          # both channel halves


@dataclass(frozen=True)
class Cfg:
    ncores: int = 8
    zb: int = 4       # planes per z/y-pass batch (1, 2 or 4)
    # engine per pooling op: "v" = DVE, "g" = GpSimd
    eng: tuple = (
        ("m2z", "v"), ("m4z", "v"), ("zt", "v"),
        ("m2y", "v"), ("m4y", "v"), ("oy", "v"),
        ("m2x", "v"), ("m4x", "v"), ("px", "v"),
    )
    # DMA trigger engines: P loads cycle through in_q, sig stores through out_q
    in_q: tuple = ("sync", "scalar")
    out_q: tuple = ("sync",)


FULL = Cfg()


def build_nc(cfg: Cfg):
    import concourse.bacc as bacc
    import concourse.tile as tile
    from concourse import mybir

    AF = mybir.ActivationFunctionType
    f32 = mybir.dt.float32
    bf16 = mybir.dt.bfloat16
    eng = dict(cfg.eng)

    nc = bacc.Bacc("TRN2", target_bir_lowering=False, debug=False,
                   enable_asserts=False, num_devices=cfg.ncores)

    pg = nc.dram_tensor("pg", [128, NX * PLANE], bf16, kind="ExternalInput").ap()
    w1 = nc.dram_tensor("w1", [C, R], bf16, kind="ExternalInput").ap()
    w2 = nc.dram_tensor("w2", [R, C], bf16, kind="ExternalInput").ap()
    out = nc.dram_tensor("out", [128, 2 * NX * VOXH], bf16,
                         kind="ExternalOutput").ap()

    def E(op):
        return nc.vector if eng[op] == "v" else nc.gpsimd

    with tile.TileContext(nc) as tc, ExitStack() as ctx:
        const = ctx.enter_context(tc.tile_pool(name="const", bufs=1))
        pp = ctx.enter_context(tc.tile_pool(name="pp", bufs=2))
        m2zp = ctx.enter_context(tc.tile_pool(name="m2zp", bufs=1))
        m4zp = ctx.enter_context(tc.tile_pool(name="m4zp", bufs=1))
        ztp = ctx.enter_context(tc.tile_pool(name="ztp", bufs=1))
        m2yp = ctx.enter_context(tc.tile_pool(name="m2yp", bufs=1))
        m4yp = ctx.enter_context(tc.tile_pool(name="m4yp", bufs=1))
        pxp = ctx.enter_context(tc.tile_pool(name="pxp", bufs=2))
        hpp = ctx.enter_context(tc.tile_pool(name="hpp", bufs=2, space="PSUM"))
        y2p = ctx.enter_context(tc.tile_pool(name="y2p", bufs=2, space="PSUM"))
        hsp = ctx.enter_context(tc.tile_pool(name="hsp", bufs=3))
        sgp = ctx.enter_context(tc.tile_pool(name="sgp", bufs=3))

        B = cfg.zb
        NB = NX // B

        neg = const.tile([128, VOX2], bf16)
        nc.gpsimd.memset(neg[:], SENT)
        w1sb = const.tile([128, 2 * R], bf16)
        nc.sync.dma_start(
            w1sb[:].rearrange("p (h r) -> p h r", h=2),
            w1.rearrange("(h p) r -> p h r", p=128),
        )
        w2sb = const.tile([128, C], bf16)
        nc.sync.dma_start(w2sb[:], w2)
        w1v = w1sb[:].rearrange("p (h r) -> p h r", h=2)

        out4 = out.rearrange("p (h x v) -> p h x v", h=2, x=NX)

        # persistent zt buffer [b, 2h, 8z, 70y] (z-major); y borders -inf once
        zt = const.tile([128, B * 2 * ZS * YP], bf16)
        ztv = zt[:].rearrange("p (b h z y) -> p b h z y", b=B, h=2, z=ZS)
        nc.gpsimd.memset(ztv[:, :, :, :, 0:3], SENT)
        nc.gpsimd.memset(ztv[:, :, :, :, YP - 3:YP], SENT)

        # x-pass ring arenas (planes keyed by index mod S*); all -inf initially
        # so out-of-range plane reads on the LEFT see -inf. Right edge handled
        # by explicit tail ops (ring slots there alias old planes).
        S0, S2, S4 = 8, 8, 12
        a_oy = const.tile([128, S0 * VOX2], bf16)
        a_m2 = const.tile([128, S2 * VOX2], bf16)
        a_m4 = const.tile([128, S4 * VOX2], bf16)
        nc.gpsimd.memset(a_oy[:], SENT)
        nc.gpsimd.memset(a_m2[:], SENT)
        nc.gpsimd.memset(a_m4[:], SENT)
        oyA = a_oy[:].rearrange("p (s v) -> p s v", s=S0)
        m2A = a_m2[:].rearrange("p (s v) -> p s v", s=S2)
        m4A = a_m4[:].rearrange("p (s v) -> p s v", s=S4)

        def chunks(j0, j1, shift_rings):
            """Split [j0, j1) so no (j+s) range crosses a multiple of S."""
            pts = set()
            for s, S in shift_rings:
                j = j0 + ((-s - j0) % S)
                while j < j1:
                    if j > j0:
                        pts.add(j)
                    j += S
            bounds = [j0] + sorted(pts) + [j1]
            return list(zip(bounds[:-1], bounds[1:]))

        def rsl(A, S, j, n):
            s = j % S
            return A[:, s:s + n, :]

        def ring_max(outA, So, so, in0A, S0_, s0, in1A, S1_, s1, j0, j1):
            for a, b in chunks(j0, j1, [(so, So), (s0, S0_), (s1, S1_)]):
                E("m2x").tensor_max(
                    rsl(outA, So, a + so, b - a),
                    rsl(in0A, S0_, a + s0, b - a),
                    rsl(in1A, S1_, a + s1, b - a),
                )

        def mlp(pxap, k):
            pxv = pxap.rearrange("p (h v) -> p h v", h=2)
            hp = hpp.tile([128, VOXH], f32, space="PSUM", tag="hp", name="hp")
            for h in (0, 1):
                nc.tensor.matmul(
                    hp[:], w1v[:, h, :], pxv[:, h, :],
                    start=(h == 0), stop=(h == 1),
                )
            hs = hsp.tile([128, VOXH], bf16, tag="hs", name="hs")
            nc.scalar.activation(hs[:], hp[:], AF.Relu)
            y2 = y2p.tile([128, VOX2], f32, space="PSUM", tag="y2", name="y2")
            y2v = y2[:].rearrange("p (h v) -> p h v", h=2)
            for h in (0, 1):
                nc.tensor.matmul(
                    y2v[:, h, :], w2sb[:, h * 128:(h + 1) * 128], hs[:],
                    start=True, stop=True,
                )
            sg = sgp.tile([128, VOX2], bf16, tag="sg", name="sg")
            nc.scalar.activation(sg[:], y2[:], AF.Sigmoid)
            qo = getattr(nc, cfg.out_q[k % len(cfg.out_q)])
            qo.dma_start(
                out4[:, :, k, :], sg[:].rearrange("p (h v) -> p h v", h=2)
            )

        def xquad(t):
            """Quad x-pass after oy batch t (planes 4t..4t+4) is in a_oy."""
            ring_max(m2A, S2, 0, oyA, S0, 0, oyA, S0, 1, 4 * t - 1, 4 * t + 3)
            ring_max(m4A, S4, 0, m2A, S2, 0, m2A, S2, 2, 4 * t - 3, 4 * t + 1)
            k0 = 4 * t - 3
            px = pxp.tile([128, B * VOX2], bf16, tag="px", name="px")
            pxA = px[:].rearrange("p (s v) -> p s v", s=B)
            for a, b in chunks(k0, k0 + 4, [(-3, S4), (0, S4)]):
                E("px").tensor_max(
                    pxA[:, a - k0:b - k0, :],
                    rsl(m4A, S4, a - 3, b - a),
                    rsl(m4A, S4, a, b - a),
                )
            for k in range(max(k0, 0), k0 + 4):
                mlp(pxA[:, k - k0, :], k)

        def xtail():
            """Planes 61..63: right-edge ops; out-of-range reads use neg."""
            M = E("m2x").tensor_max
            M(rsl(m2A, S2, NX - 1, 1), rsl(oyA, S0, NX - 1, 1), neg[:])
            M(rsl(m4A, S4, 61, 1), rsl(m2A, S2, 61, 1), rsl(m2A, S2, 63, 1))
            M(rsl(m4A, S4, 62, 1), rsl(m2A, S2, 62, 1), neg[:])
            M(rsl(m4A, S4, 63, 1), rsl(m2A, S2, 63, 1), neg[:])
            px = pxp.tile([128, 3 * VOX2], bf16, tag="px", name="px")
            pxA = px[:].rearrange("p (s v) -> p s v", s=3)
            for idx, k in enumerate(range(61, 64)):
                M(pxA[:, idx, :], rsl(m4A, S4, k - 3, 1), rsl(m4A, S4, k, 1))
                mlp(pxA[:, idx, :], k)

        for t in range(NB):
            P = pp.tile([128, B * PLANE], bf16, tag="P", name="P")
            for pl in range(B):
                q = getattr(nc, cfg.in_q[(t * B + pl) % len(cfg.in_q)])
                q.dma_start(
                    P[:, pl * PLANE:(pl + 1) * PLANE],
                    pg[:, (t * B + pl) * PLANE:(t * B + pl + 1) * PLANE],
                )
            Pv = P[:].rearrange("p (b h z y) -> p b h z y", b=B, h=2, z=ZP)
            Pc = Pv[:, :, :, :, 3:YP - 3]

            # z-pass (64 real y cols only): 16 -> 14 -> 12 -> 8
            m2z = m2zp.tile([128, B * 2 * 14 * D], bf16, tag="m2z", name="m2z")
            m2zv = m2z[:].rearrange("p (b h z y) -> p b h z y", b=B, h=2, z=14)
            E("m2z").tensor_max(m2zv, Pc[:, :, :, 0:14, :], Pc[:, :, :, 1:15, :])
            m4z = m4zp.tile([128, B * 2 * 12 * D], bf16, tag="m4z", name="m4z")
            m4zv = m4z[:].rearrange("p (b h z y) -> p b h z y", b=B, h=2, z=12)
            E("m4z").tensor_max(
                m4zv, m2zv[:, :, :, 0:12, :], m2zv[:, :, :, 2:14, :]
            )
            E("zt").tensor_max(
                ztv[:, :, :, :, 3:YP - 3],
                m4zv[:, :, :, 0:8, :], m4zv[:, :, :, 3:11, :],
            )

            # y-pass: 70 -> 69 -> 67 -> 64
            m2y = m2yp.tile([128, B * 2 * ZS * 69], bf16, tag="m2y", name="m2y")
            m2yv = m2y[:].rearrange("p (b h z y) -> p b h z y", b=B, h=2, z=ZS)
            E("m2y").tensor_max(m2yv, ztv[:, :, :, :, 0:69], ztv[:, :, :, :, 1:70])
            m4y = m4yp.tile([128, B * 2 * ZS * 67], bf16, tag="m4y", name="m4y")
            m4yv = m4y[:].rearrange("p (b h z y) -> p b h z y", b=B, h=2, z=ZS)
            E("m4y").tensor_max(
                m4yv, m2yv[:, :, :, :, 0:67], m2yv[:, :, :, :, 2:69]
            )
            s0 = (t * B) % S0
            oyv = a_oy[:].rearrange(
                "p (s h z y) -> p s h z y", s=S0, h=2, z=ZS
            )[:, s0:s0 + B]
            E("oy").tensor_max(
                oyv, m4yv[:, :, :, :, 0:64], m4yv[:, :, :, :, 3:67]
            )
            xquad(t)
        xtail()

    nc.compile()
    return nc


def host_prep(cfg: Cfg, feats, coords, W1, W2):
    """Build per-core dense pooling-layout grids + gather metadata."""
    import ml_dtypes

    bf16 = ml_dtypes.bfloat16
    featsb = np.ascontiguousarray(feats.astype(bf16))
    ix = coords[:, 0].astype(np.int64)
    iy = coords[:, 1].astype(np.int64)
    iz = coords[:, 2].astype(np.int64)
    w1h = np.ascontiguousarray(W1.astype(bf16))
    w2h = np.ascontiguousarray(W2.astype(bf16))

    in_maps, aux = [], []
    for k in range(cfg.ncores):
        zlo = k * ZS - 3
        sel = (iz >= zlo) & (iz < zlo + ZH)
        g1 = np.full((NX, ZP, YP, C), SENT, bf16)
        g1[ix[sel], iz[sel] - zlo, iy[sel] + 3] = featsb[sel]
        # (x, z, y, h, p) -> (p, x, h, z, y)
        pgk = np.ascontiguousarray(
            g1.reshape(NX, ZP, YP, 2, 128).transpose(4, 0, 3, 1, 2)
            .reshape(128, NX * PLANE)
        )
        own = np.where((iz >= k * ZS) & (iz < (k + 1) * ZS))[0]
        aux.append((own, ix[own], (iz[own] - k * ZS) * D + iy[own]))
        in_maps.append({"pg": pgk, "w1": w1h, "w2": w2h})
    return in_maps, aux


def host_post(cfg: Cfg, results, feats, aux):
    out_full = np.empty((NPTS, C), np.float32)
    for k in range(cfg.ncores):
        o = np.asarray(results[k]["out"]).reshape(128, 2, NX, VOXH)
        own, xs, vs = aux[k]
        sig = o[:, :, xs, vs]                     # (128, 2, n)
        sig = sig.transpose(2, 1, 0).reshape(len(own), C).astype(np.float32)
        out_full[own] = feats[own] * sig
    return out_full


_CACHE = {}


def _get_nc(cfg: Cfg):
    if cfg not in _CACHE:
        _CACHE[cfg] = build_nc(cfg)
    return _CACHE[cfg]


def kernel(feats, coords, W1, W2):
    from concourse.bass_utils import run_bass_kernel_spmd

    cfg = FULL
    nc = _get_nc(cfg)
    feats = np.asarray(feats, np.float32)
    in_maps, aux = host_prep(
        cfg, feats, np.asarray(coords), np.asarray(W1, np.float32),
        np.asarray(W2, np.float32),
    )
    res = run_bass_kernel_spmd(nc, in_maps, core_ids=list(range(cfg.ncores)))
    return host_post(cfg, res.results, feats, aux)
